# revision 1
# baseline (speedup 1.0000x reference)
"""GQA attention kernel for 8 Trainium2 NeuronCores.

Head-parallel sharding: core c owns q-heads [4c, 4c+4) and kv-head c.
Each core computes its 4 heads' attention and a partial output
projection (row-parallel wo); the host sums the 8 partials.

On-device layout is fully transposed: projections are computed as
w.T @ x.T so q/k/v arrive as [head_dim, tokens]; scores are computed
as s.T = [keys, queries]; the AV matmul uses [v | 1] as the stationary
operand, which also produces the softmax denominator row; the result
att.T [head_cols, tokens] feeds the output projection as the
stationary operand with no transposes anywhere (except 16 small
v-tile transposes per batch).

RoPE is applied during PSUM eviction of the projections using a
stream-shuffle pair swap plus cos/sin pattern tensors built on host.
Softmax skips max-subtraction (scores are ~N(0,1); exp cannot
overflow fp32) so only a sum (via the ones column) is needed.
"""

import sys

sys.path.insert(0, "/opt/trn_rl_repo")

import numpy as np
import ml_dtypes

import concourse.bacc as bacc
import concourse.bass as bass
import concourse.mybir as mybir
from concourse import tile
from concourse.bass_utils import run_bass_kernel_spmd
from concourse.masks import make_identity

B, T, D = 2, 2048, 2048
H, HKV, HD = 32, 8, 64
NCORE = 8
HLOC = H // NCORE          # 4 q heads per core
QCOLS = HLOC * HD          # 256
NB = T // 512              # token nblocks per batch
KC = D // 128              # contraction chunks for projections

# matmul/storage mode: "bf16" | "f32r" | "f32"
MODE = "bf16"

F32 = mybir.dt.float32
F32R = mybir.dt.float32r
BF16 = mybir.dt.bfloat16

SWAP_MASK = [i ^ 1 for i in range(32)]  # adjacent pair swap per quadrant


def _sd():
    return BF16 if MODE == "bf16" else F32


def _np_sd():
    return ml_dtypes.bfloat16 if MODE == "bf16" else np.float32


def _mm_ap(ap):
    if MODE == "f32r":
        return ap.bitcast(F32R)
    return ap


def build_nc():
    SD = _sd()
    nc = bacc.Bacc(None, target_bir_lowering=False, debug=False)

    xT = nc.dram_tensor("xT", [D, B * T], SD, kind="ExternalInput")
    wq_d = nc.dram_tensor("wq", [D, QCOLS], SD, kind="ExternalInput")
    wkv_d = nc.dram_tensor("wkv", [D, 2 * HD], SD, kind="ExternalInput")
    wo_d = nc.dram_tensor("wo", [QCOLS, D], SD, kind="ExternalInput")
    rAq_d = nc.dram_tensor("ropeAq", [128, T], SD, kind="ExternalInput")
    rBq_d = nc.dram_tensor("ropeBq", [128, T], SD, kind="ExternalInput")
    rAkv_d = nc.dram_tensor("ropeAkv", [128, T], SD, kind="ExternalInput")
    rBkv_d = nc.dram_tensor("ropeBkv", [128, T], SD, kind="ExternalInput")
    out_d = nc.dram_tensor("out", [B * T, D], F32, kind="ExternalOutput")

    with tile.TileContext(nc) as tc:
        with (
            tc.tile_pool(name="consts", bufs=1) as consts,
            tc.tile_pool(name="xp", bufs=6) as xp,
            tc.tile_pool(name="shufp", bufs=4) as shufp,
            tc.tile_pool(name="ropea", bufs=4) as ropea,
            tc.tile_pool(name="ropeb", bufs=4) as ropeb,
            tc.tile_pool(name="qropep", bufs=8 if MODE == "bf16" else 4) as qropep,
            tc.tile_pool(name="kvropep", bufs=3) as kvropep,
            tc.tile_pool(name="vextp", bufs=2) as vextp,
            tc.tile_pool(name="ptp", bufs=20) as ptp,
            tc.tile_pool(name="recipp", bufs=3) as recipp,
            tc.tile_pool(name="rbcp", bufs=3) as rbcp,
            tc.tile_pool(name="attp", bufs=4 if MODE == "bf16" else 2) as attp,
            tc.tile_pool(name="outp", bufs=6) as outp,
            tc.tile_pool(name="ps", bufs=8, space="PSUM") as psp,
        ):
            # ---- constants ----
            wq_sb = consts.tile([128, KC, QCOLS], SD)
            nc.sync.dma_start(
                out=wq_sb[:], in_=wq_d.rearrange("(kc p) m -> p kc m", p=128)
            )
            wkv_sb = consts.tile([128, KC, 2 * HD], SD)
            nc.sync.dma_start(
                out=wkv_sb[:], in_=wkv_d.rearrange("(kc p) m -> p kc m", p=128)
            )
            wo_sb = consts.tile([128, 2, D], SD)
            nc.sync.dma_start(
                out=wo_sb[:], in_=wo_d.rearrange("(g p) n -> p g n", p=128)
            )
            rAq = consts.tile([128, T], SD)
            nc.sync.dma_start(out=rAq[:], in_=rAq_d[:])
            rBq = consts.tile([128, T], SD)
            nc.sync.dma_start(out=rBq[:], in_=rBq_d[:])
            rAkv = consts.tile([128, T], SD)
            nc.sync.dma_start(out=rAkv[:], in_=rAkv_d[:])
            rBkv = consts.tile([128, T], SD)
            nc.sync.dma_start(out=rBkv[:], in_=rBkv_d[:])

            zf = consts.tile([128, 512], F32)
            nc.gpsimd.memset(zf[:], 0.0)
            ident = consts.tile([128, 128], SD)
            make_identity(nc, ident[:])
            # trimask[k, q] = 1.0 if k <= q else 0.0  (keys on partitions)
            trimask = consts.tile([128, 128], SD)
            nc.gpsimd.memset(trimask[:], 1.0)
            # keep 1.0 where -k + q >= 0, else 0
            nc.gpsimd.affine_select(
                out=trimask[:],
                in_=trimask[:],
                compare_op=mybir.AluOpType.is_ge,
                fill=0.0,
                base=0,
                pattern=[[1, 128]],
                channel_multiplier=-1,
            )

            for b in range(B):
                # ---- QKV projections + fused RoPE eviction ----
                qrope = [qropep.tile([64, T], SD, tag="qrope", name="qrope") for _ in range(4)]
                kvrope = kvropep.tile([128, T], SD, tag="kvrope", name="kvrope")
                for nb in range(NB):
                    ps = [psp.tile([128, 512], F32, tag="ps", name="ps") for _ in range(3)]
                    for kc in range(KC):
                        xt = xp.tile([128, 512], SD, tag="x", name="x")
                        c0 = b * T + nb * 512
                        nc.sync.dma_start(
                            out=xt[:], in_=xT[kc * 128 : (kc + 1) * 128, c0 : c0 + 512]
                        )
                        for mt in range(3):
                            if mt < 2:
                                lhsT = wq_sb[:, kc, mt * 128 : (mt + 1) * 128]
                            else:
                                lhsT = wkv_sb[:, kc, :]
                            nc.tensor.matmul(
                                ps[mt][:],
                                _mm_ap(lhsT),
                                _mm_ap(xt[:]),
                                start=(kc == 0),
                                stop=(kc == KC - 1),
                            )
                    for mt in range(3):
                        A = rAq if mt < 2 else rAkv
                        Bp = rBq if mt < 2 else rBkv
                        sl = slice(nb * 512, (nb + 1) * 512)
                        Asl = A[:, sl]
                        Bsl = Bp[:, sl]
                        tmp = shufp.tile([128, 512], F32, tag="shuf", name="shuf")
                        nc.vector.stream_shuffle(tmp[:], ps[mt][:], SWAP_MASK)
                        t2 = ropea.tile([128, 512], SD, tag="ra", name="ra")
                        nc.vector.tensor_mul(t2[:], ps[mt][:], Asl)
                        t3 = ropeb.tile([128, 512], SD, tag="rb", name="rb")
                        nc.vector.tensor_mul(t3[:], tmp[:], Bsl)
                        if mt < 2:
                            nc.vector.tensor_add(
                                qrope[2 * mt][:, sl], t2[0:64, :], t3[0:64, :]
                            )
                            nc.vector.tensor_add(
                                qrope[2 * mt + 1][:, sl], t2[64:128, :], t3[64:128, :]
                            )
                        else:
                            nc.vector.tensor_add(kvrope[:, sl], t2[:], t3[:])

                # ---- v transpose into [keys, 64 | ones] chunks ----
                v_ext = vextp.tile([128, KC, HD + 1], SD, tag="vext", name="vext")
                nc.gpsimd.memset(v_ext[:], 1.0)
                for j in range(KC):
                    tp = psp.tile([128, HD], SD, tag="ps", name="tpv")
                    nc.tensor.transpose(
                        tp[:],
                        kvrope[HD:128, j * 128 : (j + 1) * 128],
                        ident[HD:128, HD:128],
                    )
                    nc.scalar.activation(
                        v_ext[:, j, 0:HD],
                        tp[:],
                        mybir.ActivationFunctionType.Copy,
                    )

                # ---- attention (4 heads) ----
                attT = [attp.tile([128, T], SD, tag="attT", name="attT") for _ in range(2)]
                for h in range(HLOC):
                    qTh = qrope[h]
                    for qb in range(NB):
                        nch = 4 * qb + 4
                        q0 = qb * 512
                        av = psp.tile([128, 512], F32, tag="ps", name="av")
                        for j in range(nch):
                            sp = psp.tile([128, 512], F32, tag="ps", name="ps")
                            nc.tensor.matmul(
                                sp[:],
                                _mm_ap(kvrope[0:HD, j * 128 : (j + 1) * 128]),
                                _mm_ap(qTh[:, q0 : q0 + 512]),
                                start=True,
                                stop=True,
                            )
                            pt = ptp.tile([128, 512], SD, tag="pt", name="pt")
                            jj = j - 4 * qb
                            if jj >= 0:
                                mc = 128 * jj
                                if mc:
                                    nc.gpsimd.memset(pt[:, 0:mc], 0.0)
                                nc.scalar.activation(
                                    pt[:, mc:512],
                                    sp[:, mc:512],
                                    mybir.ActivationFunctionType.Exp,
                                )
                                nc.vector.tensor_mul(
                                    pt[:, mc : mc + 128],
                                    pt[:, mc : mc + 128],
                                    trimask[:],
                                )
                            else:
                                nc.scalar.activation(
                                    pt[:], sp[:], mybir.ActivationFunctionType.Exp
                                )
                            # AV interleaved per chunk keeps the PE stream
                            # dense (no HAM cooldown waiting on all exps)
                            nc.tensor.matmul(
                                av[0 : HD + 1, :],
                                _mm_ap(v_ext[:, j, :]),
                                _mm_ap(pt[:]),
                                start=(j == 0),
                                stop=(j == nch - 1),
                            )
                        # normalize: reciprocal of the sums row (fused into
                        # AV via the ones column), broadcast, scale
                        rc = recipp.tile([1, 512], F32, tag="recip", name="recip")
                        nc.vector.reciprocal(rc[:], av[HD : HD + 1, :])
                        rb = rbcp.tile([HD, 512], F32, tag="rbc", name="rbc")
                        nc.gpsimd.partition_broadcast(rb[:], rc[:], channels=HD)
                        dest = attT[h // 2][
                            HD * (h % 2) : HD * (h % 2) + HD, q0 : q0 + 512
                        ]
                        nc.vector.tensor_mul(dest, av[0:HD, :], rb[:])

                # ---- output projection (partial over this core's 256 cols) ----
                for mt in range(T // 128):
                    for nb2 in range(NB):
                        op = psp.tile([128, 512], F32, tag="ps", name="ps")
                        for g in range(2):
                            nc.tensor.matmul(
                                op[:],
                                _mm_ap(attT[g][:, mt * 128 : (mt + 1) * 128]),
                                _mm_ap(wo_sb[:, g, nb2 * 512 : (nb2 + 1) * 512]),
                                start=(g == 0),
                                stop=(g == 1),
                            )
                        ot = outp.tile([128, 512], F32, tag="ot", name="ot")
                        if (mt * NB + nb2) % 4 == 0:
                            nc.vector.tensor_copy(ot[:], op[:])
                        else:
                            nc.scalar.activation(
                                ot[:], op[:], mybir.ActivationFunctionType.Copy
                            )
                        r0 = b * T + mt * 128
                        nc.sync.dma_start(
                            out=out_d[r0 : r0 + 128, nb2 * 512 : (nb2 + 1) * 512],
                            in_=ot[:],
                        )

    nc.compile()
    return nc


_NC = None


def _get_nc():
    global _NC
    if _NC is None:
        _NC = build_nc()
    return _NC


def make_in_maps(x, freqs_cos, freqs_sin, wq, wk, wv, wo):
    npdt = _np_sd()
    x = np.asarray(x, np.float32)
    freqs_cos = np.asarray(freqs_cos, np.float32)
    freqs_sin = np.asarray(freqs_sin, np.float32)
    wq = np.asarray(wq, np.float32)
    wk = np.asarray(wk, np.float32)
    wv = np.asarray(wv, np.float32)
    wo = np.asarray(wo, np.float32)

    xT = np.ascontiguousarray(x.reshape(B * T, D).T.astype(npdt))

    cosT = freqs_cos.T  # [32, T]
    sinT = freqs_sin.T
    A64 = np.empty((64, T), np.float32)
    A64[0::2] = cosT
    A64[1::2] = cosT
    B64 = np.empty((64, T), np.float32)
    B64[0::2] = -sinT
    B64[1::2] = sinT
    one64 = np.ones((64, T), np.float32)
    zero64 = np.zeros((64, T), np.float32)
    rAq = np.ascontiguousarray(np.concatenate([A64, A64], 0).astype(npdt))
    rBq = np.ascontiguousarray(np.concatenate([B64, B64], 0).astype(npdt))
    rAkv = np.ascontiguousarray(np.concatenate([A64, one64], 0).astype(npdt))
    rBkv = np.ascontiguousarray(np.concatenate([B64, zero64], 0).astype(npdt))

    scale = np.float32(1.0 / np.sqrt(HD))
    in_maps = []
    for c in range(NCORE):
        wq_c = np.ascontiguousarray((wq[:, c * QCOLS : (c + 1) * QCOLS] * scale).astype(npdt))
        wkv_c = np.ascontiguousarray(
            np.concatenate(
                [wk[:, c * HD : (c + 1) * HD], wv[:, c * HD : (c + 1) * HD]], 1
            ).astype(npdt)
        )
        wo_c = np.ascontiguousarray(wo[c * QCOLS : (c + 1) * QCOLS, :].astype(npdt))
        in_maps.append(
            {
                "xT": xT,
                "wq": wq_c,
                "wkv": wkv_c,
                "wo": wo_c,
                "ropeAq": rAq,
                "ropeBq": rBq,
                "ropeAkv": rAkv,
                "ropeBkv": rBkv,
            }
        )
    return in_maps


def run(in_maps, trace=False, **kwargs):
    nc = _get_nc()
    return run_bass_kernel_spmd(
        nc, in_maps, core_ids=list(range(NCORE)), trace=trace, **kwargs
    )


def kernel(x, freqs_cos, freqs_sin, wq, wk, wv, wo):
    in_maps = make_in_maps(x, freqs_cos, freqs_sin, wq, wk, wv, wo)
    res = run(in_maps)
    total = np.zeros((B * T, D), np.float32)
    for r in res.results:
        total += r["out"]
    return total.reshape(B, T, D)

